# revision 6
# baseline (speedup 1.0000x reference)
"""DeepReservoir (leaky ESN, 4 modules) Trainium2 Bass kernel.

Problem: h[t] = (1-a)*h[t-1] + a*tanh(u[t] @ Kin + h[t-1] @ W + bias) per
module, T=8192 steps, U=1024 units, a=0.9, batch 1.  Output = all states,
modules concatenated on the feature axis: [1, T, 4*1024].

Strategy (module parallel, per the sharding hint):
  - One reservoir module per NeuronCore (4 modules; cores 4-7 run duplicates
    so one SPMD program serves all 8 cores; host gathers from cores 0-3).
  - The input projection c[t] = u[t] @ Kin + bias has no time dependence and
    is tiny (4 GFLOP total): computed on the host, shipped pre-swizzled into
    the exact per-chunk SBUF layout so the per-iteration DMA is one fully
    contiguous 128KB block (dynamic strided layouts cost ~80us/DMA in
    descriptor processing - measured).
  - The time scan is the serial bottleneck: per step a [1024]x[1024,1024]
    matvec on TensorE as 64 LDWEIGHTS+MATMUL pairs of [128,128]x[128,1]
    (weight-load bound, ~70-100ns/pair).  Weights are bf16 (enables
    fast-weight-load) with leaky a folded in: W' = a*W.  State is kept fp32
    via the rescaled recurrence h'[t] = (1-a)*h'[t-1] + tanh(W' h'[t-1] +
    c[t]); the output is a*h'.
  - Per step the matmuls are phase-ordered (contraction tiles 0-3 for all
    output tiles, then finish output tiles 0-3, then 4-7) so ScalarE/VectorE
    process the first half of the new state while TensorE finishes the
    second half, and the next step's matmuls (which need only the first
    half as contraction input) start immediately -> TensorE stays busy.
  - tanh on ScalarE (one [128,4] op per half), z+c add and leaky blends on
    VectorE; the bf16 copy of the new state is written first to unblock
    TensorE.
  - Output states are staged in SBUF and DMAd per 32-step chunk in the
    SBUF-native layout; the host inverts the layout after gathering.
"""

import numpy as np
import ml_dtypes

import concourse.bacc as bacc
import concourse.tile as tile
import concourse.mybir as mybir
from concourse.bass import ds
from concourse.bass_utils import run_bass_kernel_spmd

F32 = mybir.dt.float32
BF16 = mybir.dt.bfloat16
F8 = mybir.dt.float8e4

# Recurrent weights are shipped as fp8e4m3 scaled by W_SCALE (a power of two
# so the rescale is exact): FWL loads 4 fp8/cycle vs 2 bf16 -> the
# weight-load-bound scan runs ~2x faster.  The matmul result is divided by
# W_SCALE on VectorE when c[t] is added (same instruction count).
W_SCALE = 64.0

UNITS = 1024
IN = 64
KT = 8  # contraction tiles (1024/128)
MT = 8  # output-unit tiles (1024/128)
P = 128

LEAKY = np.float32(0.9)
ONE_MINUS_LEAKY = float(np.float32(1.0) - np.float32(0.9))

N_CORES = 8
N_MODULES = 4


def build_nc(T: int, unroll: int):
    """Build the single-core SPMD Bass program for one reservoir module."""
    assert T % unroll == 0 and unroll % 2 == 0
    nchunk = T // unroll
    nc = bacc.Bacc("TRN2", debug=False)

    wT = nc.dram_tensor("wT", [UNITS, UNITS], F8, kind="ExternalInput")
    # c pre-swizzled on host: c_in[chunk, p, s, j] = c[chunk*unroll+s, j*128+p]
    c_in = nc.dram_tensor("c_in", [nchunk, P, unroll, MT], F32, kind="ExternalInput")
    # output in SBUF-native layout: hs[chunk, p, s, j] = h[chunk*unroll+s, j*128+p]
    hs = nc.dram_tensor("hs", [nchunk, P, unroll, MT], F32, kind="ExternalOutput")

    with tile.TileContext(nc) as tc:
        with (
            tc.tile_pool(name="const", bufs=1) as const_pool,
            tc.tile_pool(name="cin", bufs=2) as cin_pool,
            tc.tile_pool(name="hout", bufs=2) as hout_pool,
            tc.tile_pool(name="work", bufs=2) as work_pool,
            tc.tile_pool(name="zpsum", bufs=2, space="PSUM") as zpsum_pool,
        ):
            # weights: w_sb[p, k, m, c] = W'[k*128+p, m*128+c]
            w_sb = const_pool.tile([P, KT, MT, P], F8)
            nc.sync.dma_start(
                w_sb[:], wT[:, :].rearrange("(k p) (m c) -> p k m c", p=P, c=P)
            )

            # persistent scan state (ping-pong on dim 1 by step parity)
            hstate = const_pool.tile([P, 2, MT], F32)  # h' fp32 master
            h16 = const_pool.tile([P, 2, MT], BF16)  # bf16 copy for PE rhs
            nc.vector.memset(hstate[:, 1, :], 0.0)
            nc.vector.memset(h16[:, 1, :], 0.0)

            c_v = c_in[:, :, :, :].rearrange("c p s j -> p c s j")
            hs_v = hs[:, :, :, :].rearrange("c p s j -> p c s j")

            with tc.For_i(
                0,
                nchunk,
                1,
                hint_engines=(mybir.EngineType.PE, mybir.EngineType.Activation),
            ) as iv:
                cchunk = cin_pool.tile([P, unroll, MT], F32, tag="cchunk")
                nc.sync.dma_start(cchunk[:], c_v[:, ds(iv, 1), :, :])
                hstage = hout_pool.tile([P, unroll, MT], F32, tag="hstage")

                for s in range(unroll):
                    cur = s % 2
                    prev = 1 - cur
                    zA = zpsum_pool.tile([P, 4], F32, tag="zA")
                    zB = zpsum_pool.tile([P, 4], F32, tag="zB")

                    def mm(k, m, start, stop):
                        zt = zA if m < 4 else zB
                        nc.tensor.matmul(
                            zt[:, (m % 4) : (m % 4) + 1],
                            w_sb[:, k, m, :],
                            h16[:, prev, k : k + 1],
                            start=start,
                            stop=stop,
                        )

                    # phase 1: contraction tiles 0-3 (only needs half A of
                    # h16, which the previous step produced early)
                    for k in range(4):
                        for m in range(MT):
                            mm(k, m, start=(k == 0 and m % 4 == 0), stop=False)
                    # phase 2a: finish z columns 0-3 so ScalarE can start
                    for m in range(4):
                        for k in range(4, 8):
                            mm(k, m, start=False, stop=(k == 7 and m == 3))
                    # phase 2b: finish z columns 4-7
                    for m in range(4, 8):
                        for k in range(4, 8):
                            mm(k, m, start=False, stop=(k == 7 and m == 7))

                    zc = work_pool.tile([P, MT], F32, tag="zc")
                    o32 = work_pool.tile([P, MT], F32, tag="o32")
                    for (lo, hi), zt in (((0, 4), zA), ((4, 8), zB)):
                        # zc = z / W_SCALE + c[t]
                        nc.vector.scalar_tensor_tensor(
                            out=zc[:, lo:hi],
                            in0=zt[:, 0:4],
                            scalar=float(1.0 / W_SCALE),
                            in1=cchunk[:, s, lo:hi],
                            op0=mybir.AluOpType.mult,
                            op1=mybir.AluOpType.add,
                        )
                        # o = tanh(zc)
                        nc.scalar.activation(
                            o32[:, lo:hi],
                            zc[:, lo:hi],
                            mybir.ActivationFunctionType.Tanh,
                        )
                        # critical-path first: bf16 state for the next matmuls
                        nc.vector.scalar_tensor_tensor(
                            out=h16[:, cur, lo:hi],
                            in0=hstate[:, prev, lo:hi],
                            scalar=ONE_MINUS_LEAKY,
                            in1=o32[:, lo:hi],
                            op0=mybir.AluOpType.mult,
                            op1=mybir.AluOpType.add,
                        )
                        # fp32 master state (off critical path)
                        nc.vector.scalar_tensor_tensor(
                            out=hstate[:, cur, lo:hi],
                            in0=hstate[:, prev, lo:hi],
                            scalar=ONE_MINUS_LEAKY,
                            in1=o32[:, lo:hi],
                            op0=mybir.AluOpType.mult,
                            op1=mybir.AluOpType.add,
                        )
                    # output h[t] = a * h'[t]
                    nc.vector.tensor_scalar_mul(
                        hstage[:, s, :], hstate[:, cur, :], float(LEAKY)
                    )

                nc.sync.dma_start(hs_v[:, ds(iv, 1), :, :], hstage[:])

    nc.compile()
    return nc


def _prep_in_maps(u, kernel, rec_kernel, bias, T, unroll):
    nchunk = T // unroll
    u0 = np.asarray(u[0], dtype=np.float32)  # [T, 64]
    in_maps = []
    for core in range(N_CORES):
        m = core % N_MODULES
        wT = np.ascontiguousarray(
            (np.asarray(rec_kernel[m], dtype=np.float32) * LEAKY * W_SCALE).astype(
                ml_dtypes.float8_e4m3
            )
        )
        # c[t, u] = u[t] @ Kin + bias  (fp32, host)
        c = u0 @ np.asarray(kernel[m], dtype=np.float32) + np.asarray(
            bias[m], dtype=np.float32
        )
        # -> c_in[chunk, p, s, j]
        c_sw = np.ascontiguousarray(
            c.reshape(nchunk, unroll, MT, P).transpose(0, 3, 1, 2)
        )
        in_maps.append({"wT": wT, "c_in": c_sw})
    return in_maps


def _unswizzle(hs_dev, T, unroll):
    # hs_dev[chunk, p, s, j] -> [T, 1024] with unit u = j*128+p
    nchunk = T // unroll
    return np.ascontiguousarray(
        hs_dev.transpose(0, 2, 3, 1).reshape(T, UNITS)
    )


_NC_CACHE = {}


def run(u, kernel, rec_kernel, bias, unroll=32, trace=False):
    T = u.shape[1]
    key = (T, unroll)
    if key not in _NC_CACHE:
        _NC_CACHE[key] = build_nc(T, unroll)
    nc = _NC_CACHE[key]
    in_maps = _prep_in_maps(u, kernel, rec_kernel, bias, T, unroll)
    res = run_bass_kernel_spmd(
        nc, in_maps, core_ids=list(range(N_CORES)), trace=trace
    )
    out = np.concatenate(
        [_unswizzle(res.results[m]["hs"], T, unroll) for m in range(N_MODULES)],
        axis=1,
    )  # [T, 4096]
    return out[None].astype(np.float32), res


def kernel(u, kernel, rec_kernel, bias):
    out, _ = run(u, kernel, rec_kernel, bias)
    return out



# revision 8
# speedup vs baseline: 10.3639x; 10.3639x over previous
"""DeepReservoir (leaky ESN, 4 modules) Trainium2 Bass kernel.

Problem: h[t] = (1-a)*h[t-1] + a*tanh(u[t] @ Kin + h[t-1] @ W + bias) per
module, T=8192 steps, U=1024 units, a=0.9, batch 1.  Output = all states,
modules concatenated on the feature axis: [1, T, 4*1024].

Strategy:
  - The scan is serial and the per-step matvec costs 64 self-loading
    [128,128]x[128,N] matmuls on TensorE, each pinned at a ~53ns
    per-instruction floor (dispatch + weight load) regardless of N (N<=64)
    and of weight dtype.  So the win is amortizing MORE time steps per
    instruction, not faster weight loads.
  - The reservoir has strongly fading memory (leaky a=0.9 and tanh heavily
    saturated by the input drive): a scan started from h=0 converges to the
    true trajectory to ~1e-4 within ~32 steps.  Each module's timeline is
    therefore split into 2*S segments of L=T/(2*S) steps, each segment
    recomputed from h=0 with a WARM=64-step warmup window (segment 0's
    warmup uses c=0, which keeps h exactly at the 0 fixed point, so it
    stays bit-exact).
  - Module m runs on cores m (segments 0..S-1) and m+4 (segments S..2S-1).
    Within a core the S segments run INTERLEAVED: one [128,128]x[128,S]
    matmul advances all S segments one step -> the 64-instruction step cost
    covers S time steps.  No inter-core communication (remote DMA is
    unsupported in this environment; the duplicate cores previously wasted).
  - c[t] = u[t] @ Kin + bias precomputed on host (tiny), shipped pre-swizzled
    per chunk so the per-chunk DMA is one contiguous block.
  - Weights bf16 with leaky folded in (W' = a*W); state master fp32 via the
    rescaled recurrence h' = (1-a)h' + tanh(W'h' + c); output h = a*h'.
  - Per step matmuls are phase-ordered (contraction chunks 0-3 for all
    m-tiles, then finish m-tiles 0-3, then 4-7) so ScalarE/VectorE process
    the first half of the new state while TensorE finishes the second half.
  - Output states staged in SBUF, DMAd per chunk in SBUF-native layout;
    host drops each segment's warmup rows and reassembles the timeline.
"""

import numpy as np
import ml_dtypes

import concourse.bacc as bacc
import concourse.tile as tile
import concourse.mybir as mybir
from concourse.bass import ds
from concourse.bass_utils import run_bass_kernel_spmd

F32 = mybir.dt.float32
BF16 = mybir.dt.bfloat16

UNITS = 1024
IN = 64
KT = 8  # contraction tiles (1024/128)
MT = 8  # output-unit tiles (1024/128)
P = 128

LEAKY = np.float32(0.9)
ONE_MINUS_LEAKY = float(np.float32(1.0) - np.float32(0.9))

N_CORES = 8
N_MODULES = 4
SEG = 8  # interleaved segments per core (matmul free dim N)
WARM = 64  # warmup steps per segment (fading-memory reconvergence)


def build_nc(T: int, unroll: int, seg: int = SEG, warm: int = WARM):
    """Single-core SPMD program: `seg` interleaved scan segments of one
    reservoir module, T_seg = T//(2*seg) + warm steps each."""
    L = T // (2 * seg)
    t_seg = L + warm
    assert t_seg % unroll == 0 and unroll % 2 == 0 and warm % unroll == 0
    nchunk = t_seg // unroll
    nc = bacc.Bacc("TRN2", debug=False)

    wT = nc.dram_tensor("wT", [UNITS, UNITS], BF16, kind="ExternalInput")
    # c pre-swizzled on host:
    # c_in[chunk, p, s, j, g] = c_seg[g][chunk*unroll+s, j*128+p]
    c_in = nc.dram_tensor(
        "c_in", [nchunk, P, unroll, MT, seg], F32, kind="ExternalInput"
    )
    # output, SBUF-native: hs[chunk, p, s, j, g] = h_seg[g][chunk*unroll+s, j*128+p]
    hs = nc.dram_tensor(
        "hs", [nchunk, P, unroll, MT, seg], F32, kind="ExternalOutput"
    )

    with tile.TileContext(nc) as tc:
        with (
            tc.tile_pool(name="const", bufs=1) as const_pool,
            tc.tile_pool(name="cin", bufs=2) as cin_pool,
            tc.tile_pool(name="hout", bufs=2) as hout_pool,
            tc.tile_pool(name="work", bufs=2) as work_pool,
            tc.tile_pool(name="zpsum", bufs=2, space="PSUM") as zpsum_pool,
        ):
            # weights: w_sb[p, k, m, c] = W'[k*128+p, m*128+c]
            w_sb = const_pool.tile([P, KT, MT, P], BF16)
            nc.sync.dma_start(
                w_sb[:], wT[:, :].rearrange("(k p) (m c) -> p k m c", p=P, c=P)
            )

            # persistent scan state (ping-pong on dim 1 by step parity)
            hstate = const_pool.tile([P, 2, MT, seg], F32)  # h' fp32 master
            h16 = const_pool.tile([P, 2, MT, seg], BF16)  # bf16 copy, PE rhs
            nc.vector.memset(hstate[:, 1, :, :], 0.0)
            nc.vector.memset(h16[:, 1, :, :], 0.0)

            c_v = c_in[:, :, :, :, :].rearrange("c p s j g -> p c s j g")
            hs_v = hs[:, :, :, :, :].rearrange("c p s j g -> p c s j g")

            with tc.For_i(
                0,
                nchunk,
                1,
                hint_engines=(mybir.EngineType.PE, mybir.EngineType.Activation),
            ) as iv:
                cchunk = cin_pool.tile([P, unroll, MT, seg], F32, tag="cchunk")
                nc.sync.dma_start(cchunk[:], c_v[:, ds(iv, 1), :, :, :])
                hstage = hout_pool.tile([P, unroll, MT, seg], F32, tag="hstage")

                for s in range(unroll):
                    cur = s % 2
                    prev = 1 - cur
                    zA = zpsum_pool.tile([P, 4, seg], F32, tag="zA")
                    zB = zpsum_pool.tile([P, 4, seg], F32, tag="zB")

                    def mm(k, m, start, stop):
                        zt = zA if m < 4 else zB
                        nc.tensor.matmul(
                            zt[:, (m % 4), :],
                            w_sb[:, k, m, :],
                            h16[:, prev, k, :],
                            start=start,
                            stop=stop,
                        )

                    # phase 1: contraction chunks 0-3 (only needs half A of
                    # h16, which the previous step produced early)
                    for k in range(4):
                        for m in range(MT):
                            mm(k, m, start=(k == 0 and m % 4 == 0), stop=False)
                    # phase 2a: finish z m-tiles 0-3 so ScalarE can start
                    for m in range(4):
                        for k in range(4, 8):
                            mm(k, m, start=False, stop=(k == 7 and m == 3))
                    # phase 2b: finish z m-tiles 4-7
                    for m in range(4, 8):
                        for k in range(4, 8):
                            mm(k, m, start=False, stop=(k == 7 and m == 7))

                    zc = work_pool.tile([P, MT, seg], F32, tag="zc")
                    o32 = work_pool.tile([P, MT, seg], F32, tag="o32")
                    for (lo, hi), zt in (((0, 4), zA), ((4, 8), zB)):
                        # zc = z + c[t]
                        nc.vector.tensor_add(
                            zc[:, lo:hi, :], zt[:, 0:4, :], cchunk[:, s, lo:hi, :]
                        )
                        # o = tanh(zc)
                        nc.scalar.activation(
                            o32[:, lo:hi, :],
                            zc[:, lo:hi, :],
                            mybir.ActivationFunctionType.Tanh,
                        )
                        # critical-path first: bf16 state for the next matmuls
                        nc.vector.scalar_tensor_tensor(
                            out=h16[:, cur, lo:hi, :],
                            in0=hstate[:, prev, lo:hi, :],
                            scalar=ONE_MINUS_LEAKY,
                            in1=o32[:, lo:hi, :],
                            op0=mybir.AluOpType.mult,
                            op1=mybir.AluOpType.add,
                        )
                        # fp32 master state (off critical path)
                        nc.vector.scalar_tensor_tensor(
                            out=hstate[:, cur, lo:hi, :],
                            in0=hstate[:, prev, lo:hi, :],
                            scalar=ONE_MINUS_LEAKY,
                            in1=o32[:, lo:hi, :],
                            op0=mybir.AluOpType.mult,
                            op1=mybir.AluOpType.add,
                        )
                    # output h[t] = a * h'[t]
                    nc.vector.tensor_scalar_mul(
                        hstage[:, s, :, :], hstate[:, cur, :, :], float(LEAKY)
                    )

                nc.sync.dma_start(hs_v[:, ds(iv, 1), :, :, :], hstage[:])

    nc.compile()
    return nc


def _prep_in_maps(u, kernel, rec_kernel, bias, T, unroll, seg=SEG, warm=WARM):
    L = T // (2 * seg)
    t_seg = L + warm
    nchunk = t_seg // unroll
    u0 = np.asarray(u[0], dtype=np.float32)  # [T, 64]
    in_maps = []
    for core in range(N_CORES):
        m = core % N_MODULES
        half = core // N_MODULES  # 0: segments 0..seg-1, 1: seg..2*seg-1
        wT = np.ascontiguousarray(
            (np.asarray(rec_kernel[m], dtype=np.float32) * LEAKY).astype(
                ml_dtypes.bfloat16
            )
        )
        # c[t, u] = u[t] @ Kin + bias  (fp32, host)
        c = u0 @ np.asarray(kernel[m], dtype=np.float32) + np.asarray(
            bias[m], dtype=np.float32
        )
        # per-segment window [sigma*L - warm, (sigma+1)*L), zero-padded
        # before t=0 (c=0 keeps h exactly at the 0 fixed point)
        cs = np.zeros((t_seg, UNITS, seg), np.float32)
        for g in range(seg):
            sigma = half * seg + g
            t0 = sigma * L - warm
            lo = max(t0, 0)
            cs[lo - t0 :, :, g] = c[lo : t0 + t_seg]
        # -> c_in[chunk, p, s, j, g]
        c_sw = np.ascontiguousarray(
            cs.reshape(nchunk, unroll, MT, P, seg).transpose(0, 3, 1, 2, 4)
        )
        in_maps.append({"wT": wT, "c_in": c_sw})
    return in_maps


def _assemble(hs_by_core, T, unroll, seg=SEG, warm=WARM):
    """hs_by_core: list of 8 arrays [nchunk, P, unroll, MT, seg] ->
    full output [T, 4*UNITS]."""
    L = T // (2 * seg)
    t_seg = L + warm
    nchunk = t_seg // unroll
    out = np.empty((T, N_MODULES * UNITS), np.float32)
    for core in range(N_CORES):
        m = core % N_MODULES
        half = core // N_MODULES
        hs_dev = hs_by_core[core]
        # [nchunk, P, unroll, MT, seg] -> [t_seg, UNITS, seg]
        flat = hs_dev.transpose(0, 2, 3, 1, 4).reshape(t_seg, UNITS, seg)
        for g in range(seg):
            sigma = half * seg + g
            out[sigma * L : (sigma + 1) * L, m * UNITS : (m + 1) * UNITS] = flat[
                warm:, :, g
            ]
    return out


_NC_CACHE = {}


def run(u, kernel, rec_kernel, bias, unroll=32, seg=SEG, warm=WARM, trace=False):
    T = u.shape[1]
    key = (T, unroll, seg, warm)
    if key not in _NC_CACHE:
        _NC_CACHE[key] = build_nc(T, unroll, seg, warm)
    nc = _NC_CACHE[key]
    in_maps = _prep_in_maps(u, kernel, rec_kernel, bias, T, unroll, seg, warm)
    res = run_bass_kernel_spmd(
        nc, in_maps, core_ids=list(range(N_CORES)), trace=trace
    )
    out = _assemble(
        [res.results[c]["hs"] for c in range(N_CORES)], T, unroll, seg, warm
    )
    return out[None].astype(np.float32), res


def kernel(u, kernel, rec_kernel, bias):
    out, _ = run(u, kernel, rec_kernel, bias)
    return out
